# revision 5
# baseline (speedup 1.0000x reference)
"""Trainium2 Bass kernel for nn_CAML_53240414601378.

Embedding lookup -> Conv1d(k=4, pad=2) -> tanh -> per-label attention
pooling -> logits. Data-parallel over batch across 8 NeuronCores
(4 batches per core); small params replicated.

v2 pipeline design (from perfetto analysis of v1):
- Conv tiles are 508 output columns wide so each tile's x-window
  [t0-2, t0+510) is exactly one 512-index gather chunk (chunk j starts
  at 508j-2). A 128-index tail chunk covers the last tile. Every conv
  matmul group therefore depends on ONE gather chunk -> the PE starts
  ~15us in instead of waiting ~54us for the whole first batch.
- Gathers are spread round-robin over 4 SWDGE queues (chained only
  within a queue to keep per-ring FIFO drain order); v1 ran one queue
  at ~63 GB/s which serialized the whole 8 MB gather behind the PE.
- idx table is uploaded via the HWDGE (sync) path, shared num_idxs
  registers avoid 36 serial GpSimd MOVEs, logits DMA out per batch.
- scores matmul for tile j-1 is emitted after conv tile j so the PE
  never waits on the tanh activation of the tile it just produced.
"""

import numpy as np
import ml_dtypes

import concourse.bass as bass
import concourse.tile as tile
from concourse.tile import add_dep_helper
from concourse import bacc, mybir
from concourse.bass_utils import run_bass_kernel_spmd

B, S = 32, 4096
VOCAB, E, F, L = 30522, 256, 256, 50
SO = S + 1  # conv output length (4097)
N_CORES = 8
BPC = B // N_CORES  # batches per core
BF16 = mybir.dt.bfloat16
FP32 = mybir.dt.float32

TW = 508                      # conv tile width (outputs per tile)
NJ = (SO + TW - 1) // TW      # 9 tiles: 8x508 + 33
NQ = 1                        # SWDGE queues for the gathers

# gather chunk j covers x cols [CLO[j], CLO[j]+CSZ[j]) and feeds conv
# tile j (outputs [508j, 508j+n), x-window [508j-2, 508j+510))
CLO = [0] + [TW * j - 2 for j in range(1, 8)] + [S - 128]
CSZ = [512] * 8 + [128]
IPB = sum(CSZ) // 16          # idx cols per batch (4224/16 = 264)

_cache = {}


def _tile_n(j):
    return min(SO - TW * j, TW)


def _conv_mms(j):
    """Shift pieces for tile j: (k, lo, hi, off) with x cols [lo, hi)
    from chunk j, psum offset off; widest first."""
    t0, n = TW * j, _tile_n(j)
    shifts = []
    for k in range(4):
        lo = max(0, t0 + k - 2)
        hi = min(S, t0 + k - 2 + n)
        shifts.append((k, lo, hi, lo - (t0 + k - 2)))
    shifts.sort(key=lambda s: -(s[2] - s[1]))
    return shifts


def build_nc():
    nc = bacc.Bacc("TRN2", target_bir_lowering=False, debug=False,
                   num_devices=N_CORES, num_swdge_queues=NQ)

    emb_ap = nc.dram_tensor("emb", (VOCAB, E), BF16, kind="ExternalInput").ap()
    idx_ap = nc.dram_tensor("idx", (128, BPC * IPB), mybir.dt.int16,
                            kind="ExternalInput").ap()
    w_ap = nc.dram_tensor("wconv", (128, 16, 128), BF16,
                          kind="ExternalInput").ap()
    uw_ap = nc.dram_tensor("uwfw", (128, 2, 114), BF16,
                           kind="ExternalInput").ap()
    cb_ap = nc.dram_tensor("cbias", (128, 2), FP32, kind="ExternalInput").ap()
    fb_ap = nc.dram_tensor("fbias", (L, 1), FP32, kind="ExternalInput").ap()
    out_ap = nc.dram_tensor("out", (L, BPC), FP32, kind="ExternalOutput").ap()

    with tile.TileContext(nc) as tc:
        with (
            tc.tile_pool(name="const", bufs=1) as const,
            tc.tile_pool(name="xh", bufs=2) as xh,      # gather chunk tiles
            tc.tile_pool(name="hp", bufs=2) as hp,
            tc.tile_pool(name="ep", bufs=3) as ep,      # exp scratch tiles
            tc.tile_pool(name="pp", bufs=2) as pp,      # per-batch partials
            tc.tile_pool(name="small", bufs=8) as small,
            tc.tile_pool(name="psum", bufs=2, space="PSUM") as psum,
            tc.tile_pool(name="psum_st", bufs=4, space="PSUM") as psum_st,
        ):
            # ---- constants; idx first so gathers start ASAP ----
            idx_sb = const.tile([128, BPC * IPB], mybir.dt.int16)
            nc.sync.dma_start(idx_sb[:], idx_ap[:])
            w_sb = const.tile([128, 16, 128], BF16)
            nc.sync.dma_start(w_sb[:], w_ap[:])
            uw_sb = const.tile([128, 2, 114], BF16)
            nc.sync.dma_start(uw_sb[:], uw_ap[:])
            cb_sb = const.tile([128, 2], FP32)
            nc.sync.dma_start(cb_sb[:], cb_ap[:])
            fb_sb = const.tile([L, 1], FP32)
            nc.sync.dma_start(fb_sb[:], fb_ap[:])
            out_sb = const.tile([L, BPC], FP32)

            r512 = nc.gpsimd.to_reg(512)
            r128 = nc.gpsimd.to_reg(128)

            prev_q = [None] * NQ  # last gather per queue (ring FIFO order)
            qrr = [0]             # round-robin cursor

            def gather_chunk(b, j, xs):
                cs = CSZ[j]
                c0 = sum(CSZ[:j]) // 16
                q = qrr[0] % NQ
                qrr[0] += 1
                gi = nc.gpsimd.dma_gather(
                    out_ap=xs[:], in_ap=emb_ap[:],
                    idxs_ap=idx_sb[:, b * IPB + c0: b * IPB + c0 + cs // 16],
                    num_idxs=cs, num_idxs_reg=(r512 if cs == 512 else r128),
                    elem_size=E, transpose=True, single_packet=False,
                    queue_num=q)
                if prev_q[q] is not None:
                    # NB add_dep_helper(from, to) = "from depends on to"
                    add_dep_helper(gi.ins, prev_q[q].ins, False,
                                   "gather ring order")
                prev_q[q] = gi

            for b in range(BPC):
                xs = []
                for j in range(NJ):
                    xt = xh.tile([128, 2, CSZ[j]], BF16, tag=f"c{j}")
                    gather_chunk(b, j, xt)
                    xs.append(xt)

                H = hp.tile([128, 2, SO], BF16, tag="H")
                nmx = pp.tile([L, NJ], FP32, tag="nmx")  # -max per tile
                zp = pp.tile([L, NJ], FP32, tag="zp")    # partial Z
                np_ = pp.tile([L, NJ], FP32, tag="np")   # partial num

                def score_tile(j):
                    """Combined scores/t matmul for H tile j and the
                    online-softmax partials for that tile."""
                    t0, n = TW * j, _tile_n(j)
                    pst = psum_st.tile([114, TW], FP32, tag="st")
                    for fc in range(2):
                        nc.tensor.matmul(
                            pst[:, 0:n], uw_sb[:, fc, :], H[:, fc, t0:t0 + n],
                            start=(fc == 0), stop=(fc == 1),
                        )
                    nc.vector.reduce_max(nmx[:, j:j + 1], pst[0:L, 0:n],
                                         axis=mybir.AxisListType.X,
                                         negate=True)
                    e_sb = ep.tile([L, TW], FP32, tag="e")
                    nc.scalar.activation(
                        e_sb[:, 0:n], pst[0:L, 0:n],
                        mybir.ActivationFunctionType.Exp,
                        bias=nmx[:, j:j + 1], accum_out=zp[:, j:j + 1],
                    )
                    nc.vector.tensor_mul(e_sb[:, 0:n], e_sb[:, 0:n],
                                         pst[64:64 + L, 0:n])
                    nc.vector.reduce_sum(np_[:, j:j + 1], e_sb[:, 0:n],
                                         axis=mybir.AxisListType.X)

                # ---- conv1d(k=4) + bias + tanh; scores lag one tile ----
                for j in range(NJ):
                    t0, n = TW * j, _tile_n(j)
                    for fc in range(2):
                        ph = psum.tile([128, TW], FP32, tag=f"h{fc}")
                        mms = [(k, lo, hi, off, ec)
                               for (k, lo, hi, off) in _conv_mms(j)
                               for ec in range(2)]
                        for i, (k, lo, hi, off, ec) in enumerate(mms):
                            rel = lo - CLO[j]
                            nc.tensor.matmul(
                                ph[:, off:off + (hi - lo)],
                                w_sb[:, k * 4 + ec * 2 + fc, :],
                                xs[j][:, ec, rel:rel + (hi - lo)],
                                start=(i == 0), stop=(i == len(mms) - 1),
                            )
                        nc.scalar.activation(
                            H[:, fc, t0:t0 + n], ph[:, 0:n],
                            mybir.ActivationFunctionType.Tanh,
                            bias=cb_sb[:, fc:fc + 1],
                        )
                    if j > 0:
                        score_tile(j - 1)
                score_tile(NJ - 1)

                # ---- combine partials -> logits ----
                nm = small.tile([L, 1], FP32, tag="nm")  # -(global max)
                nc.vector.reduce_max(nm[:], nmx[:], axis=mybir.AxisListType.X,
                                     op=mybir.AluOpType.min)
                wj = small.tile([L, NJ], FP32, tag="wj")
                nc.scalar.activation(
                    wj[:], nmx[:], mybir.ActivationFunctionType.Exp,
                    bias=nm[:], scale=-1.0,
                )
                wz = small.tile([L, NJ], FP32, tag="wz")
                nc.vector.tensor_mul(wz[:], wj[:], zp[:])
                zsum = small.tile([L, 1], FP32, tag="zsum")
                nc.vector.reduce_sum(zsum[:], wz[:], axis=mybir.AxisListType.X)
                nc.vector.tensor_mul(wj[:], wj[:], np_[:])
                nsum = small.tile([L, 1], FP32, tag="nsum")
                nc.vector.reduce_sum(nsum[:], wj[:], axis=mybir.AxisListType.X)
                zr = small.tile([L, 1], FP32, tag="zr")
                nc.vector.reciprocal(zr[:], zsum[:])
                sm = small.tile([L, 1], FP32, tag="sm")
                nc.vector.tensor_mul(sm[:], nsum[:], zr[:])
                nc.vector.tensor_add(out_sb[:, b:b + 1], sm[:], fb_sb[:])
                nc.sync.dma_start(out_ap[:, b:b + 1], out_sb[:, b:b + 1])

    nc.compile()
    return nc


def _prep_shared(emb_table, conv_w, conv_b, U_w, final_w, final_b):
    emb_bf = np.ascontiguousarray(emb_table.astype(ml_dtypes.bfloat16))

    # wconv[e_lo, k*4 + ec*2 + fc, f_lo] = conv_w[fc*128+f, ec*128+e, k]
    W = np.empty((128, 16, 128), np.float32)
    for k in range(4):
        for ec in range(2):
            for fc in range(2):
                W[:, k * 4 + ec * 2 + fc, :] = conv_w[
                    fc * 128:(fc + 1) * 128, ec * 128:(ec + 1) * 128, k].T
    W = np.ascontiguousarray(W.astype(ml_dtypes.bfloat16))

    # uwfw[f_lo, fc, j]: j<50 -> U_w[j, fc*128+f_lo];
    # j in [64,114) -> final_w[j-64, fc*128+f_lo]; rest zero
    UW = np.zeros((128, 2, 114), np.float32)
    UW[:, :, 0:L] = U_w.T.reshape(2, 128, L).transpose(1, 0, 2)
    UW[:, :, 64:64 + L] = final_w.T.reshape(2, 128, L).transpose(1, 0, 2)
    UW = np.ascontiguousarray(UW.astype(ml_dtypes.bfloat16))

    CB = np.ascontiguousarray(conv_b.reshape(2, 128).T.astype(np.float32))
    FB = np.ascontiguousarray(final_b.reshape(L, 1).astype(np.float32))
    return emb_bf, W, UW, CB, FB


def kernel(input_ids, emb_table, conv_w, conv_b, U_w, final_w, final_b):
    import os
    ids = np.asarray(input_ids)
    emb_table = np.asarray(emb_table, dtype=np.float32)
    conv_w = np.asarray(conv_w, dtype=np.float32)
    conv_b = np.asarray(conv_b, dtype=np.float32)
    U_w = np.asarray(U_w, dtype=np.float32)
    final_w = np.asarray(final_w, dtype=np.float32)
    final_b = np.asarray(final_b, dtype=np.float32)

    if "nc" not in _cache:
        _cache["nc"] = build_nc()
    nc = _cache["nc"]

    emb_bf, W, UW, CB, FB = _prep_shared(
        emb_table, conv_w, conv_b, U_w, final_w, final_b)

    ids16 = ids.astype(np.int16)  # vocab 30522 < 2**15
    in_maps = []
    for c in range(N_CORES):
        cid = ids16[c * BPC:(c + 1) * BPC]  # (BPC, S)
        # per-chunk index blocks: chunk pos p -> [p % 16, coloff + p // 16];
        # the 16-row block is replicated to all 8 gpsimd cores (128 rows)
        cols = []
        for b in range(BPC):
            for j in range(NJ):
                v = cid[b, CLO[j]:CLO[j] + CSZ[j]]
                cols.append(v.reshape(CSZ[j] // 16, 16).T)
        blk = np.concatenate(cols, axis=1)  # (16, BPC*IPB)
        idx = np.tile(blk, (8, 1))
        in_maps.append({
            "emb": emb_bf, "idx": np.ascontiguousarray(idx),
            "wconv": W, "uwfw": UW, "cbias": CB, "fbias": FB,
        })

    trace = bool(int(os.environ.get("KERNEL_TRACE", "0")))
    res = run_bass_kernel_spmd(nc, in_maps, core_ids=list(range(N_CORES)),
                               trace=trace)
    _cache["last_result"] = res

    out = np.concatenate(
        [res.results[c]["out"].T for c in range(N_CORES)], axis=0)
    return np.ascontiguousarray(out.astype(np.float32))


# revision 18
# speedup vs baseline: 1.3434x; 1.3434x over previous
"""Trainium2 Bass kernel for nn_CAML_53240414601378.

Embedding lookup -> Conv1d(k=4, pad=2) -> tanh -> per-label attention
pooling -> logits. Data-parallel over batch across 8 NeuronCores
(4 batches per core); small params replicated.

v3 pipeline design (from perfetto analysis of v1/v2):
- Conv tiles are ~508 output columns wide so each tile's x-window
  [t0-2, t0+510) is exactly one 512-index gather chunk. Every conv
  matmul group depends on ONE gather chunk, so the PE starts as soon
  as the first chunk lands instead of waiting for a whole batch.
  Batch 0 uses a small 128-index first chunk (124-col first tile) to
  start the PE even earlier.
- Gathers run on 4 SWDGE queues. Tile assigns DMA-completion sem
  lanes round-robin over the 8 DMASW lanes in *scheduled* order with
  cumulative thresholds, which silently assumes per-lane in-order
  completion; with multiple queues that only holds if lane index and
  queue index stay congruent. All 36 gathers are therefore chained
  with ordering-only deps (pinning scheduled order = emission order)
  and queue = position % 4, so lane g%8 always serves queue g%4.
- Warmup matmuls (consumed by a dummy reduce) run during the initial
  gather latency so the PE HAM clock gate is already at 8/8 when the
  real conv stream starts.
- scores matmul for tile j-1 is emitted after conv tile j so the PE
  never waits on the tanh activation of the tile it just produced.
"""

import numpy as np
import ml_dtypes

import concourse.bass as bass
import concourse.tile as tile
from concourse.tile import add_dep_helper
from concourse import bacc, mybir
from concourse.bass_utils import run_bass_kernel_spmd

B, S = 32, 4096
VOCAB, E, F, L = 30522, 256, 256, 50
SO = S + 1  # conv output length (4097)
N_CORES = 8
BPC = B // N_CORES  # batches per core
BF16 = mybir.dt.bfloat16
FP32 = mybir.dt.float32

TW = 508                      # max conv tile width (outputs per tile)
NJ = 9                        # tiles per batch
NQ = 4                        # SWDGE queues for the gathers


def _plan(b):
    """Per-batch tile plan: list of (t0, n, chunk_lo, chunk_sz)."""
    if b == 0:
        p = [(0, 124, 0, 128)]
        for i in range(7):
            p.append((124 + 508 * i, 508, 122 + 508 * i, 512))
        p.append((3680, 417, 3584, 512))
        return p
    clo = [0] + [TW * j - 2 for j in range(1, 8)] + [S - 128]
    csz = [512] * 8 + [128]
    return [(TW * j, min(SO - TW * j, TW), clo[j], csz[j]) for j in range(NJ)]


IPB = sum(c[3] for c in _plan(1)) // 16  # idx cols per batch (264)

_cache = {}


def build_nc():
    nc = bacc.Bacc("TRN2", target_bir_lowering=False, debug=False,
                   num_devices=N_CORES, num_swdge_queues=NQ)

    # One semaphore per gather chunk, allocated before the TileContext so
    # they sit in a contiguous range below the tile framework's own sems.
    # Tile's auto-generated DMA waits are pruned against the *scheduling
    # sim's* timeline, which models SWDGE DMA as serialized; on hardware
    # the 4-queue gathers complete later relative to the PE stream and the
    # pruned waits let conv matmuls read chunks before the DMA lands
    # (observed as corrupted early batches). Explicit per-chunk sems +
    # tensor-engine wait_ge gates make the RAW edge unconditional; one
    # sem per chunk (16 incs, one per SDMA engine) is also safe against
    # inter-engine skew, unlike cumulative per-lane thresholds.
    gsems = [nc.alloc_semaphore(f"gch{g}") for g in range(BPC * NJ)]
    gsem_lo = min(s.num for s in gsems)
    gsem_hi = max(s.num for s in gsems)

    emb_ap = nc.dram_tensor("emb", (VOCAB, E), BF16, kind="ExternalInput").ap()
    idx_ap = nc.dram_tensor("idx", (128, BPC * IPB), mybir.dt.int16,
                            kind="ExternalInput").ap()
    w_ap = nc.dram_tensor("wconv", (128, 16, 128), BF16,
                          kind="ExternalInput").ap()
    uw_ap = nc.dram_tensor("uwfw", (128, 2, 114), BF16,
                           kind="ExternalInput").ap()
    cb_ap = nc.dram_tensor("cbias", (128, 2), FP32, kind="ExternalInput").ap()
    fb_ap = nc.dram_tensor("fbias", (L, 1), FP32, kind="ExternalInput").ap()
    out_ap = nc.dram_tensor("out", (L, BPC), FP32, kind="ExternalOutput").ap()

    with tile.TileContext(nc) as tc:
        with (
            tc.tile_pool(name="const", bufs=1) as const,
            # 4 bufs: each batch gets its own buffer per chunk tag, so no
            # gather ever overwrites a buffer an earlier batch still reads
            # (WAR edges across batches disappear structurally)
            tc.tile_pool(name="xh", bufs=4) as xh,      # gather chunk tiles
            tc.tile_pool(name="hp", bufs=2) as hp,
            tc.tile_pool(name="ep", bufs=3) as ep,      # exp scratch tiles
            tc.tile_pool(name="pp", bufs=2) as pp,      # per-batch partials
            tc.tile_pool(name="small", bufs=8) as small,
            tc.tile_pool(name="psum", bufs=2, space="PSUM") as psum,
            tc.tile_pool(name="psum_st", bufs=4, space="PSUM") as psum_st,
        ):
            # ---- constants; idx first so gathers start ASAP ----
            idx_sb = const.tile([128, BPC * IPB], mybir.dt.int16)
            nc.sync.dma_start(idx_sb[:], idx_ap[:])
            w_sb = const.tile([128, 16, 128], BF16)
            nc.sync.dma_start(w_sb[:], w_ap[:])
            uw_sb = const.tile([128, 2, 114], BF16)
            nc.sync.dma_start(uw_sb[:], uw_ap[:])
            cb_sb = const.tile([128, 2], FP32)
            nc.sync.dma_start(cb_sb[:], cb_ap[:])
            fb_sb = const.tile([L, 1], FP32)
            nc.sync.dma_start(fb_sb[:], fb_ap[:])
            out_sb = const.tile([L, BPC], FP32)

            r512 = nc.gpsimd.to_reg(512)
            r128 = nc.gpsimd.to_reg(128)

            # sems are not zeroed on alloc; clear before any gather can inc
            clr = nc.gpsimd.sem_clear(range(gsem_lo, gsem_hi + 1))

            # ---- HAM warmup: ~3.5us of throwaway matmuls so the PE
            # clock gate is 8/8 by the time the first chunk lands ----
            warm_rhs = idx_sb[:, 0:TW].bitcast(BF16)
            wp = psum_st.tile([114, TW], FP32, tag="st")
            last_pe = [None]
            for i in range(8):
                last_pe[0] = nc.tensor.matmul(
                    wp[:, 0:TW], w_sb[:, i, 0:114], warm_rhs,
                    start=(i == 0), stop=(i == 7))
            warm_out = small.tile([L, 1], FP32, tag="warm")
            nc.vector.reduce_max(warm_out[:], wp[0:L, :],
                                 axis=mybir.AxisListType.X)

            prev_g = [None]  # last gather emitted (global chain)
            gcnt = [0]

            def gather_chunk(b, coloff, csz, xs):
                g = gcnt[0]
                q = g % NQ
                gcnt[0] += 1
                gi = nc.gpsimd.dma_gather(
                    out_ap=xs[:], in_ap=emb_ap[:],
                    idxs_ap=idx_sb[:, b * IPB + coloff:
                                   b * IPB + coloff + csz // 16],
                    num_idxs=csz, num_idxs_reg=(r512 if csz == 512 else r128),
                    elem_size=E, transpose=True, single_packet=True,
                    queue_num=q)
                gi.then_inc(gsems[g], 16)
                if prev_g[0] is None:
                    add_dep_helper(gi.ins, clr.ins, False, "sems cleared")
                else:
                    # ordering-only dep: pins scheduled order = emission
                    # order so DMASW lane g%8 always serves queue g%4
                    add_dep_helper(gi.ins, prev_g[0].ins, False,
                                   "gather lane/queue alignment")
                prev_g[0] = gi

            for b in range(BPC):
                plan = _plan(b)
                xs = []
                coloff = 0
                for j, (t0, n, clo, csz) in enumerate(plan):
                    xt = xh.tile([128, 2, csz], BF16, tag=f"c{j}")
                    gather_chunk(b, coloff, csz, xt)
                    coloff += csz // 16
                    xs.append(xt)

                H = hp.tile([128, 2, SO], BF16, tag="H")
                nmx = pp.tile([L, NJ], FP32, tag="nmx")  # -max per tile
                zp = pp.tile([L, NJ], FP32, tag="zp")    # partial Z
                np_ = pp.tile([L, NJ], FP32, tag="np")   # partial num

                def score_tile(j):
                    """Combined scores/t matmul for H tile j and the
                    online-softmax partials for that tile."""
                    t0, n, _, _ = plan[j]
                    pst = psum_st.tile([114, TW], FP32, tag="st")
                    for fc in range(2):
                        last_pe[0] = nc.tensor.matmul(
                            pst[:, 0:n], uw_sb[:, fc, :], H[:, fc, t0:t0 + n],
                            start=(fc == 0), stop=(fc == 1),
                        )
                    nc.vector.reduce_max(nmx[:, j:j + 1], pst[0:L, 0:n],
                                         axis=mybir.AxisListType.X,
                                         negate=True)
                    e_sb = ep.tile([L, TW], FP32, tag="e")
                    nc.scalar.activation(
                        e_sb[:, 0:n], pst[0:L, 0:n],
                        mybir.ActivationFunctionType.Exp,
                        bias=nmx[:, j:j + 1], accum_out=zp[:, j:j + 1],
                    )
                    nc.vector.tensor_mul(e_sb[:, 0:n], e_sb[:, 0:n],
                                         pst[64:64 + L, 0:n])
                    nc.vector.reduce_sum(np_[:, j:j + 1], e_sb[:, 0:n],
                                         axis=mybir.AxisListType.X)

                # ---- conv1d(k=4) + bias + tanh; scores lag one tile ----
                for j, (t0, n, clo, csz) in enumerate(plan):
                    shifts = []
                    for k in range(4):
                        lo = max(0, t0 + k - 2)
                        hi = min(S, t0 + k - 2 + n)
                        shifts.append((k, lo, hi, lo - (t0 + k - 2)))
                    shifts.sort(key=lambda s: -(s[2] - s[1]))
                    for fc in range(2):
                        ph = psum.tile([128, TW], FP32, tag=f"h{fc}")
                        mms = [(k, lo, hi, off, ec)
                               for (k, lo, hi, off) in shifts
                               for ec in range(2)]
                        for i, (k, lo, hi, off, ec) in enumerate(mms):
                            rel = lo - clo
                            mi = nc.tensor.matmul(
                                ph[:, off:off + (hi - lo)],
                                w_sb[:, k * 4 + ec * 2 + fc, :],
                                xs[j][:, ec, rel:rel + (hi - lo)],
                                start=(i == 0), stop=(i == len(mms) - 1),
                            )
                            if i == 0 and fc == 0:
                                # explicit RAW gate, attached directly to
                                # the first matmul of the tile: PE blocks
                                # until chunk (b, j)'s gather DMA fully
                                # landed (16 incs = all 16 SDMA engines)
                                mi._wait_ge(gsems[b * NJ + j], 16)
                            last_pe[0] = mi
                        nc.scalar.activation(
                            H[:, fc, t0:t0 + n], ph[:, 0:n],
                            mybir.ActivationFunctionType.Tanh,
                            bias=cb_sb[:, fc:fc + 1],
                        )
                    if j > 0:
                        score_tile(j - 1)
                score_tile(NJ - 1)

                # ---- combine partials -> logits ----
                nm = small.tile([L, 1], FP32, tag="nm")  # -(global max)
                nc.vector.reduce_max(nm[:], nmx[:], axis=mybir.AxisListType.X,
                                     op=mybir.AluOpType.min)
                wj = small.tile([L, NJ], FP32, tag="wj")
                nc.scalar.activation(
                    wj[:], nmx[:], mybir.ActivationFunctionType.Exp,
                    bias=nm[:], scale=-1.0,
                )
                wz = small.tile([L, NJ], FP32, tag="wz")
                nc.vector.tensor_mul(wz[:], wj[:], zp[:])
                zsum = small.tile([L, 1], FP32, tag="zsum")
                nc.vector.reduce_sum(zsum[:], wz[:], axis=mybir.AxisListType.X)
                nc.vector.tensor_mul(wj[:], wj[:], np_[:])
                nsum = small.tile([L, 1], FP32, tag="nsum")
                nc.vector.reduce_sum(nsum[:], wj[:], axis=mybir.AxisListType.X)
                zr = small.tile([L, 1], FP32, tag="zr")
                nc.vector.reciprocal(zr[:], zsum[:])
                sm = small.tile([L, 1], FP32, tag="sm")
                nc.vector.tensor_mul(sm[:], nsum[:], zr[:])
                nc.vector.tensor_add(out_sb[:, b:b + 1], sm[:], fb_sb[:])

            nc.sync.dma_start(out_ap[:], out_sb[:])

    nc.compile()
    return nc


def _prep_shared(emb_table, conv_w, conv_b, U_w, final_w, final_b):
    emb_bf = np.ascontiguousarray(emb_table.astype(ml_dtypes.bfloat16))

    # wconv[e_lo, k*4 + ec*2 + fc, f_lo] = conv_w[fc*128+f, ec*128+e, k]
    W = np.empty((128, 16, 128), np.float32)
    for k in range(4):
        for ec in range(2):
            for fc in range(2):
                W[:, k * 4 + ec * 2 + fc, :] = conv_w[
                    fc * 128:(fc + 1) * 128, ec * 128:(ec + 1) * 128, k].T
    W = np.ascontiguousarray(W.astype(ml_dtypes.bfloat16))

    # uwfw[f_lo, fc, j]: j<50 -> U_w[j, fc*128+f_lo];
    # j in [64,114) -> final_w[j-64, fc*128+f_lo]; rest zero
    UW = np.zeros((128, 2, 114), np.float32)
    UW[:, :, 0:L] = U_w.T.reshape(2, 128, L).transpose(1, 0, 2)
    UW[:, :, 64:64 + L] = final_w.T.reshape(2, 128, L).transpose(1, 0, 2)
    UW = np.ascontiguousarray(UW.astype(ml_dtypes.bfloat16))

    CB = np.ascontiguousarray(conv_b.reshape(2, 128).T.astype(np.float32))
    FB = np.ascontiguousarray(final_b.reshape(L, 1).astype(np.float32))
    return emb_bf, W, UW, CB, FB


def _build_idx(cid):
    """(BPC, S) int16 -> (128, BPC*IPB) gather index table."""
    cols = []
    for b in range(BPC):
        for (t0, n, clo, csz) in _plan(b):
            v = cid[b, clo:clo + csz]
            cols.append(v.reshape(csz // 16, 16).T)
    blk = np.concatenate(cols, axis=1)  # (16, BPC*IPB)
    return np.ascontiguousarray(np.tile(blk, (8, 1)))


def kernel(input_ids, emb_table, conv_w, conv_b, U_w, final_w, final_b):
    import os
    ids = np.asarray(input_ids)
    emb_table = np.asarray(emb_table, dtype=np.float32)
    conv_w = np.asarray(conv_w, dtype=np.float32)
    conv_b = np.asarray(conv_b, dtype=np.float32)
    U_w = np.asarray(U_w, dtype=np.float32)
    final_w = np.asarray(final_w, dtype=np.float32)
    final_b = np.asarray(final_b, dtype=np.float32)

    if "nc" not in _cache:
        _cache["nc"] = build_nc()
    nc = _cache["nc"]

    emb_bf, W, UW, CB, FB = _prep_shared(
        emb_table, conv_w, conv_b, U_w, final_w, final_b)

    ids16 = ids.astype(np.int16)  # vocab 30522 < 2**15
    in_maps = []
    for c in range(N_CORES):
        idx = _build_idx(ids16[c * BPC:(c + 1) * BPC])
        in_maps.append({
            "emb": emb_bf, "idx": idx,
            "wconv": W, "uwfw": UW, "cbias": CB, "fbias": FB,
        })

    trace = bool(int(os.environ.get("KERNEL_TRACE", "0")))
    res = run_bass_kernel_spmd(nc, in_maps, core_ids=list(range(N_CORES)),
                               trace=trace)
    _cache["last_result"] = res

    out = np.concatenate(
        [res.results[c]["out"].T for c in range(N_CORES)], axis=0)
    return np.ascontiguousarray(out.astype(np.float32))


# revision 20
# speedup vs baseline: 1.3507x; 1.0055x over previous
"""Trainium2 Bass kernel for nn_CAML_53240414601378.

Embedding lookup -> Conv1d(k=4, pad=2) -> tanh -> per-label attention
pooling -> logits. Data-parallel over batch across 8 NeuronCores
(4 batches per core); small params replicated.

v3 pipeline design (from perfetto analysis of v1/v2):
- Conv tiles are ~508 output columns wide so each tile's x-window
  [t0-2, t0+510) is exactly one 512-index gather chunk. Every conv
  matmul group depends on ONE gather chunk, so the PE starts as soon
  as the first chunk lands instead of waiting for a whole batch.
  Batch 0 uses a small 128-index first chunk (124-col first tile) to
  start the PE even earlier.
- Gathers run on 4 SWDGE queues. Tile assigns DMA-completion sem
  lanes round-robin over the 8 DMASW lanes in *scheduled* order with
  cumulative thresholds, which silently assumes per-lane in-order
  completion; with multiple queues that only holds if lane index and
  queue index stay congruent. All 36 gathers are therefore chained
  with ordering-only deps (pinning scheduled order = emission order)
  and queue = position % 4, so lane g%8 always serves queue g%4.
- Warmup matmuls (consumed by a dummy reduce) run during the initial
  gather latency so the PE HAM clock gate is already at 8/8 when the
  real conv stream starts.
- scores matmul for tile j-1 is emitted after conv tile j so the PE
  never waits on the tanh activation of the tile it just produced.
"""

import numpy as np
import ml_dtypes

import concourse.bass as bass
import concourse.tile as tile
from concourse.tile import add_dep_helper
from concourse import bacc, mybir
from concourse.bass_utils import run_bass_kernel_spmd

B, S = 32, 4096
VOCAB, E, F, L = 30522, 256, 256, 50
SO = S + 1  # conv output length (4097)
N_CORES = 8
BPC = B // N_CORES  # batches per core
BF16 = mybir.dt.bfloat16
FP32 = mybir.dt.float32

TW = 508                      # max conv tile width (outputs per tile)
NJ = 9                        # tiles per batch
NQ = 4                        # SWDGE queues for the gathers


def _plan(b):
    """Per-batch tile plan: list of (t0, n, chunk_lo, chunk_sz)."""
    if b == 0:
        p = [(0, 124, 0, 128)]
        for i in range(7):
            p.append((124 + 508 * i, 508, 122 + 508 * i, 512))
        p.append((3680, 417, 3584, 512))
        return p
    clo = [0] + [TW * j - 2 for j in range(1, 8)] + [S - 128]
    csz = [512] * 8 + [128]
    return [(TW * j, min(SO - TW * j, TW), clo[j], csz[j]) for j in range(NJ)]


IPB = sum(c[3] for c in _plan(1)) // 16  # idx cols per batch (264)

_cache = {}


def build_nc():
    nc = bacc.Bacc("TRN2", target_bir_lowering=False, debug=False,
                   num_devices=N_CORES, num_swdge_queues=NQ)

    # One semaphore per gather chunk, allocated before the TileContext so
    # they sit in a contiguous range below the tile framework's own sems.
    # Tile's auto-generated DMA waits are pruned against the *scheduling
    # sim's* timeline, which models SWDGE DMA as serialized; on hardware
    # the 4-queue gathers complete later relative to the PE stream and the
    # pruned waits let conv matmuls read chunks before the DMA lands
    # (observed as corrupted early batches). Explicit per-chunk sems +
    # tensor-engine wait_ge gates make the RAW edge unconditional; one
    # sem per chunk (16 incs, one per SDMA engine) is also safe against
    # inter-engine skew, unlike cumulative per-lane thresholds.
    gsems = [nc.alloc_semaphore(f"gch{g}") for g in range(BPC * NJ)]
    gsem_lo = min(s.num for s in gsems)
    gsem_hi = max(s.num for s in gsems)

    emb_ap = nc.dram_tensor("emb", (VOCAB, E), BF16, kind="ExternalInput").ap()
    idx_ap = nc.dram_tensor("idx", (128, BPC * IPB), mybir.dt.int16,
                            kind="ExternalInput").ap()
    w_ap = nc.dram_tensor("wconv", (128, 16, 128), BF16,
                          kind="ExternalInput").ap()
    uw_ap = nc.dram_tensor("uwfw", (128, 2, 114), BF16,
                           kind="ExternalInput").ap()
    cb_ap = nc.dram_tensor("cbias", (128, 2), FP32, kind="ExternalInput").ap()
    fb_ap = nc.dram_tensor("fbias", (L, 1), FP32, kind="ExternalInput").ap()
    out_ap = nc.dram_tensor("out", (L, BPC), FP32, kind="ExternalOutput").ap()

    with tile.TileContext(nc) as tc:
        with (
            tc.tile_pool(name="const", bufs=1) as const,
            # 4 bufs: each batch gets its own buffer per chunk tag, so no
            # gather ever overwrites a buffer an earlier batch still reads
            # (WAR edges across batches disappear structurally)
            tc.tile_pool(name="xh", bufs=4) as xh,      # gather chunk tiles
            tc.tile_pool(name="hp", bufs=2) as hp,
            tc.tile_pool(name="ep", bufs=3) as ep,      # exp scratch tiles
            tc.tile_pool(name="pp", bufs=2) as pp,      # per-batch partials
            tc.tile_pool(name="small", bufs=8) as small,
            tc.tile_pool(name="psum", bufs=2, space="PSUM") as psum,
            tc.tile_pool(name="psum_st", bufs=4, space="PSUM") as psum_st,
        ):
            # ---- constants; idx first so gathers start ASAP ----
            idx_sb = const.tile([128, BPC * IPB], mybir.dt.int16)
            nc.sync.dma_start(idx_sb[:], idx_ap[:])
            w_sb = const.tile([128, 16, 128], BF16)
            nc.sync.dma_start(w_sb[:], w_ap[:])
            uw_sb = const.tile([128, 2, 114], BF16)
            nc.sync.dma_start(uw_sb[:], uw_ap[:])
            cb_sb = const.tile([128, 2], FP32)
            nc.sync.dma_start(cb_sb[:], cb_ap[:])
            fb_sb = const.tile([L, 1], FP32)
            nc.sync.dma_start(fb_sb[:], fb_ap[:])
            out_sb = const.tile([L, BPC], FP32)

            r512 = nc.gpsimd.to_reg(512)
            r128 = nc.gpsimd.to_reg(128)

            # sems are not zeroed on alloc; clear before any gather can inc
            clr = nc.gpsimd.sem_clear(range(gsem_lo, gsem_hi + 1))

            # ---- HAM warmup: ~3.5us of throwaway matmuls so the PE
            # clock gate is 8/8 by the time the first chunk lands ----
            warm_rhs = idx_sb[:, 0:TW].bitcast(BF16)
            wp = psum_st.tile([114, TW], FP32, tag="st")
            last_pe = [None]
            for i in range(8):
                last_pe[0] = nc.tensor.matmul(
                    wp[:, 0:TW], w_sb[:, i, 0:114], warm_rhs,
                    start=(i == 0), stop=(i == 7))
            warm_out = small.tile([L, 1], FP32, tag="warm")
            nc.vector.reduce_max(warm_out[:], wp[0:L, :],
                                 axis=mybir.AxisListType.X)

            prev_g = [None]  # last gather emitted (global chain)
            gcnt = [0]

            def gather_chunk(b, coloff, csz, xs):
                g = gcnt[0]
                q = g % NQ
                gcnt[0] += 1
                gi = nc.gpsimd.dma_gather(
                    out_ap=xs[:], in_ap=emb_ap[:],
                    idxs_ap=idx_sb[:, b * IPB + coloff:
                                   b * IPB + coloff + csz // 16],
                    num_idxs=csz, num_idxs_reg=(r512 if csz == 512 else r128),
                    elem_size=E, transpose=True, single_packet=True,
                    queue_num=q)
                gi.then_inc(gsems[g], 16)
                if prev_g[0] is None:
                    add_dep_helper(gi.ins, clr.ins, False, "sems cleared")
                else:
                    # ordering-only dep: pins scheduled order = emission
                    # order so DMASW lane g%8 always serves queue g%4
                    add_dep_helper(gi.ins, prev_g[0].ins, False,
                                   "gather lane/queue alignment")
                prev_g[0] = gi

            for b in range(BPC):
                plan = _plan(b)
                xs = []
                coloff = 0
                for j, (t0, n, clo, csz) in enumerate(plan):
                    xt = xh.tile([128, 2, csz], BF16, tag=f"c{j}")
                    gather_chunk(b, coloff, csz, xt)
                    coloff += csz // 16
                    xs.append(xt)

                H = hp.tile([128, 2, SO], BF16, tag="H")
                zp = pp.tile([L, NJ], FP32, tag="zp")    # partial Z
                np_ = pp.tile([L, NJ], FP32, tag="np")   # partial num

                def score_tile(j):
                    """Combined scores/t matmul for H tile j and the
                    softmax partials for that tile. Scores are bounded
                    (|score| <= |U_l|·|H_s| ~ 2.6, H in [-1,1]) so raw
                    exp in fp32 needs no max subtraction."""
                    t0, n, _, _ = plan[j]
                    pst = psum_st.tile([114, TW], FP32, tag="st")
                    for fc in range(2):
                        last_pe[0] = nc.tensor.matmul(
                            pst[:, 0:n], uw_sb[:, fc, :], H[:, fc, t0:t0 + n],
                            start=(fc == 0), stop=(fc == 1),
                        )
                    e_sb = ep.tile([L, TW], FP32, tag="e")
                    nc.scalar.activation(
                        e_sb[:, 0:n], pst[0:L, 0:n],
                        mybir.ActivationFunctionType.Exp,
                        accum_out=zp[:, j:j + 1],
                    )
                    nc.vector.tensor_mul(e_sb[:, 0:n], e_sb[:, 0:n],
                                         pst[64:64 + L, 0:n])
                    nc.vector.reduce_sum(np_[:, j:j + 1], e_sb[:, 0:n],
                                         axis=mybir.AxisListType.X)

                # ---- conv1d(k=4) + bias + tanh; scores lag one tile ----
                for j, (t0, n, clo, csz) in enumerate(plan):
                    shifts = []
                    for k in range(4):
                        lo = max(0, t0 + k - 2)
                        hi = min(S, t0 + k - 2 + n)
                        shifts.append((k, lo, hi, lo - (t0 + k - 2)))
                    shifts.sort(key=lambda s: -(s[2] - s[1]))
                    for fc in range(2):
                        ph = psum.tile([128, TW], FP32, tag=f"h{fc}")
                        mms = [(k, lo, hi, off, ec)
                               for (k, lo, hi, off) in shifts
                               for ec in range(2)]
                        for i, (k, lo, hi, off, ec) in enumerate(mms):
                            rel = lo - clo
                            mi = nc.tensor.matmul(
                                ph[:, off:off + (hi - lo)],
                                w_sb[:, k * 4 + ec * 2 + fc, :],
                                xs[j][:, ec, rel:rel + (hi - lo)],
                                start=(i == 0), stop=(i == len(mms) - 1),
                            )
                            if i == 0 and fc == 0:
                                # explicit RAW gate, attached directly to
                                # the first matmul of the tile: PE blocks
                                # until chunk (b, j)'s gather DMA fully
                                # landed (16 incs = all 16 SDMA engines)
                                mi._wait_ge(gsems[b * NJ + j], 16)
                            last_pe[0] = mi
                        nc.scalar.activation(
                            H[:, fc, t0:t0 + n], ph[:, 0:n],
                            mybir.ActivationFunctionType.Tanh,
                            bias=cb_sb[:, fc:fc + 1],
                        )
                    if j > 0:
                        score_tile(j - 1)
                score_tile(NJ - 1)

                # ---- combine partials -> logits ----
                zsum = small.tile([L, 1], FP32, tag="zsum")
                nc.vector.reduce_sum(zsum[:], zp[:], axis=mybir.AxisListType.X)
                nsum = small.tile([L, 1], FP32, tag="nsum")
                nc.vector.reduce_sum(nsum[:], np_[:], axis=mybir.AxisListType.X)
                zr = small.tile([L, 1], FP32, tag="zr")
                nc.vector.reciprocal(zr[:], zsum[:])
                sm = small.tile([L, 1], FP32, tag="sm")
                nc.vector.tensor_mul(sm[:], nsum[:], zr[:])
                nc.vector.tensor_add(out_sb[:, b:b + 1], sm[:], fb_sb[:])

            nc.sync.dma_start(out_ap[:], out_sb[:])

    nc.compile()
    return nc


def _prep_shared(emb_table, conv_w, conv_b, U_w, final_w, final_b):
    emb_bf = np.ascontiguousarray(emb_table.astype(ml_dtypes.bfloat16))

    # wconv[e_lo, k*4 + ec*2 + fc, f_lo] = conv_w[fc*128+f, ec*128+e, k]
    W = np.empty((128, 16, 128), np.float32)
    for k in range(4):
        for ec in range(2):
            for fc in range(2):
                W[:, k * 4 + ec * 2 + fc, :] = conv_w[
                    fc * 128:(fc + 1) * 128, ec * 128:(ec + 1) * 128, k].T
    W = np.ascontiguousarray(W.astype(ml_dtypes.bfloat16))

    # uwfw[f_lo, fc, j]: j<50 -> U_w[j, fc*128+f_lo];
    # j in [64,114) -> final_w[j-64, fc*128+f_lo]; rest zero
    UW = np.zeros((128, 2, 114), np.float32)
    UW[:, :, 0:L] = U_w.T.reshape(2, 128, L).transpose(1, 0, 2)
    UW[:, :, 64:64 + L] = final_w.T.reshape(2, 128, L).transpose(1, 0, 2)
    UW = np.ascontiguousarray(UW.astype(ml_dtypes.bfloat16))

    CB = np.ascontiguousarray(conv_b.reshape(2, 128).T.astype(np.float32))
    FB = np.ascontiguousarray(final_b.reshape(L, 1).astype(np.float32))
    return emb_bf, W, UW, CB, FB


def _build_idx(cid):
    """(BPC, S) int16 -> (128, BPC*IPB) gather index table."""
    cols = []
    for b in range(BPC):
        for (t0, n, clo, csz) in _plan(b):
            v = cid[b, clo:clo + csz]
            cols.append(v.reshape(csz // 16, 16).T)
    blk = np.concatenate(cols, axis=1)  # (16, BPC*IPB)
    return np.ascontiguousarray(np.tile(blk, (8, 1)))


def kernel(input_ids, emb_table, conv_w, conv_b, U_w, final_w, final_b):
    import os
    ids = np.asarray(input_ids)
    emb_table = np.asarray(emb_table, dtype=np.float32)
    conv_w = np.asarray(conv_w, dtype=np.float32)
    conv_b = np.asarray(conv_b, dtype=np.float32)
    U_w = np.asarray(U_w, dtype=np.float32)
    final_w = np.asarray(final_w, dtype=np.float32)
    final_b = np.asarray(final_b, dtype=np.float32)

    if "nc" not in _cache:
        _cache["nc"] = build_nc()
    nc = _cache["nc"]

    emb_bf, W, UW, CB, FB = _prep_shared(
        emb_table, conv_w, conv_b, U_w, final_w, final_b)

    ids16 = ids.astype(np.int16)  # vocab 30522 < 2**15
    in_maps = []
    for c in range(N_CORES):
        idx = _build_idx(ids16[c * BPC:(c + 1) * BPC])
        in_maps.append({
            "emb": emb_bf, "idx": idx,
            "wconv": W, "uwfw": UW, "cbias": CB, "fbias": FB,
        })

    trace = bool(int(os.environ.get("KERNEL_TRACE", "0")))
    res = run_bass_kernel_spmd(nc, in_maps, core_ids=list(range(N_CORES)),
                               trace=trace)
    _cache["last_result"] = res

    out = np.concatenate(
        [res.results[c]["out"].T for c in range(N_CORES)], axis=0)
    return np.ascontiguousarray(out.astype(np.float32))


# revision 26
# speedup vs baseline: 1.3635x; 1.0095x over previous
"""Trainium2 Bass kernel for nn_CAML_53240414601378.

Embedding lookup -> Conv1d(k=4, pad=2) -> tanh -> per-label attention
pooling -> logits. Data-parallel over batch across 8 NeuronCores
(4 batches per core); small params replicated.

v3 pipeline design (from perfetto analysis of v1/v2):
- Conv tiles are ~508 output columns wide so each tile's x-window
  [t0-2, t0+510) is exactly one 512-index gather chunk. Every conv
  matmul group depends on ONE gather chunk, so the PE starts as soon
  as the first chunk lands instead of waiting for a whole batch.
  Batch 0 uses a small 128-index first chunk (124-col first tile) to
  start the PE even earlier.
- Gathers run on 4 SWDGE queues. Tile assigns DMA-completion sem
  lanes round-robin over the 8 DMASW lanes in *scheduled* order with
  cumulative thresholds, which silently assumes per-lane in-order
  completion; with multiple queues that only holds if lane index and
  queue index stay congruent. All 36 gathers are therefore chained
  with ordering-only deps (pinning scheduled order = emission order)
  and queue = position % 4, so lane g%8 always serves queue g%4.
- Warmup matmuls (consumed by a dummy reduce) run during the initial
  gather latency so the PE HAM clock gate is already at 8/8 when the
  real conv stream starts.
- scores matmul for tile j-1 is emitted after conv tile j so the PE
  never waits on the tanh activation of the tile it just produced.
"""

import numpy as np
import ml_dtypes

import concourse.bass as bass
import concourse.tile as tile
from concourse.tile import add_dep_helper
from concourse import bacc, mybir
from concourse.bass_utils import run_bass_kernel_spmd

B, S = 32, 4096
VOCAB, E, F, L = 30522, 256, 256, 50
SO = S + 1  # conv output length (4097)
N_CORES = 8
BPC = B // N_CORES  # batches per core
BF16 = mybir.dt.bfloat16
FP32 = mybir.dt.float32

TW = 508                      # max conv tile width (outputs per tile)
NJ = 9                        # tiles per batch
NQ = 4                        # SWDGE queues for the gathers


def _plan(b):
    """Per-batch tile plan: list of (t0, n, chunk_lo, chunk_sz)."""
    if b == 0:
        # small 128-idx chunks on all 4 queues first: the PE starts on
        # tile 0 ~4us sooner and never stalls on the gather ramp
        p = [(124 * i, 124, max(0, 124 * i - 2), 128) for i in range(4)]
        for i in range(7):
            p.append((496 + 508 * i, 508, 494 + 508 * i, 512))
        p.append((4052, 45, S - 128, 128))
        return p
    clo = [0] + [TW * j - 2 for j in range(1, 8)] + [S - 128]
    csz = [512] * 8 + [128]
    return [(TW * j, min(SO - TW * j, TW), clo[j], csz[j]) for j in range(NJ)]


IPB = sum(c[3] for c in _plan(1)) // 16  # idx cols per batch (264)
N_CHUNKS = len(_plan(0)) + (BPC - 1) * len(_plan(1))

_cache = {}


def build_nc():
    nc = bacc.Bacc("TRN2", target_bir_lowering=False, debug=False,
                   num_devices=N_CORES, num_swdge_queues=NQ)

    # One semaphore per gather chunk, allocated before the TileContext so
    # they sit in a contiguous range below the tile framework's own sems.
    # Tile's auto-generated DMA waits are pruned against the *scheduling
    # sim's* timeline, which models SWDGE DMA as serialized; on hardware
    # the 4-queue gathers complete later relative to the PE stream and the
    # pruned waits let conv matmuls read chunks before the DMA lands
    # (observed as corrupted early batches). Explicit per-chunk sems +
    # tensor-engine wait_ge gates make the RAW edge unconditional; one
    # sem per chunk (16 incs, one per SDMA engine) is also safe against
    # inter-engine skew, unlike cumulative per-lane thresholds.
    gsems = [nc.alloc_semaphore(f"gch{g}") for g in range(N_CHUNKS)]
    gsem_lo = min(s.num for s in gsems)
    gsem_hi = max(s.num for s in gsems)

    emb_ap = nc.dram_tensor("emb", (VOCAB, E), BF16, kind="ExternalInput").ap()
    idx_ap = nc.dram_tensor("idx", (128, BPC * IPB), mybir.dt.int16,
                            kind="ExternalInput").ap()
    w_ap = nc.dram_tensor("wconv", (128, 16, 128), BF16,
                          kind="ExternalInput").ap()
    uw_ap = nc.dram_tensor("uwfw", (128, 2, 114), BF16,
                           kind="ExternalInput").ap()
    cb_ap = nc.dram_tensor("cbias", (128, 2), FP32, kind="ExternalInput").ap()
    fb_ap = nc.dram_tensor("fbias", (L, 1), FP32, kind="ExternalInput").ap()
    out_ap = nc.dram_tensor("out", (L, BPC), FP32, kind="ExternalOutput").ap()

    with tile.TileContext(nc) as tc:
        with (
            tc.tile_pool(name="const", bufs=1) as const,
            # 4 bufs: each batch gets its own buffer per chunk tag, so no
            # gather ever overwrites a buffer an earlier batch still reads
            # (WAR edges across batches disappear structurally)
            tc.tile_pool(name="xh", bufs=4) as xh,      # gather chunk tiles
            tc.tile_pool(name="hp", bufs=2) as hp,
            tc.tile_pool(name="ep", bufs=3) as ep,      # exp scratch tiles
            tc.tile_pool(name="pp", bufs=2) as pp,      # per-batch partials
            tc.tile_pool(name="small", bufs=8) as small,
            tc.tile_pool(name="psum", bufs=2, space="PSUM") as psum,
            tc.tile_pool(name="psum_st", bufs=4, space="PSUM") as psum_st,
        ):
            # ---- constants; idx first so gathers start ASAP ----
            idx_sb = const.tile([128, BPC * IPB], mybir.dt.int16)
            nc.sync.dma_start(idx_sb[:], idx_ap[:])
            w_sb = const.tile([128, 16, 128], BF16)
            nc.sync.dma_start(w_sb[:], w_ap[:])
            uw_sb = const.tile([128, 2, 114], BF16)
            nc.sync.dma_start(uw_sb[:], uw_ap[:])
            cb_sb = const.tile([128, 2], FP32)
            nc.sync.dma_start(cb_sb[:], cb_ap[:])
            fb_sb = const.tile([L, 1], FP32)
            nc.sync.dma_start(fb_sb[:], fb_ap[:])
            out_sb = const.tile([L, BPC], FP32)

            r512 = nc.gpsimd.to_reg(512)
            r128 = nc.gpsimd.to_reg(128)

            # sems are not zeroed on alloc; clear before any gather can inc
            clr = nc.gpsimd.sem_clear(range(gsem_lo, gsem_hi + 1))

            # ---- HAM warmup: ~3.5us of throwaway matmuls so the PE
            # clock gate is 8/8 by the time the first chunk lands ----
            warm_rhs = idx_sb[:, 0:TW].bitcast(BF16)
            wp = psum_st.tile([114, TW], FP32, tag="st")
            last_pe = [None]
            for i in range(16):
                last_pe[0] = nc.tensor.matmul(
                    wp[:, 0:TW], w_sb[:, i % 16, 0:114], warm_rhs,
                    start=(i == 0), stop=(i == 15))
            warm_out = small.tile([L, 1], FP32, tag="warm")
            nc.vector.reduce_max(warm_out[:], wp[0:L, :],
                                 axis=mybir.AxisListType.X)

            prev_g = [None]  # last gather emitted (global chain)
            gcnt = [0]

            def gather_chunk(b, coloff, csz, xs):
                g = gcnt[0]
                q = g % NQ
                gcnt[0] += 1
                gi = nc.gpsimd.dma_gather(
                    out_ap=xs[:], in_ap=emb_ap[:],
                    idxs_ap=idx_sb[:, b * IPB + coloff:
                                   b * IPB + coloff + csz // 16],
                    num_idxs=csz, num_idxs_reg=(r512 if csz == 512 else r128),
                    elem_size=E, transpose=True, single_packet=True,
                    queue_num=q)
                gi.then_inc(gsems[g], 16)
                if prev_g[0] is None:
                    add_dep_helper(gi.ins, clr.ins, False, "sems cleared")
                else:
                    # ordering-only dep: pins scheduled order = emission
                    # order so DMASW lane g%8 always serves queue g%4
                    add_dep_helper(gi.ins, prev_g[0].ins, False,
                                   "gather lane/queue alignment")
                prev_g[0] = gi

            for b in range(BPC):
                plan = _plan(b)
                npl = len(plan)
                gbase = gcnt[0]
                xs = []
                coloff = 0
                for j, (t0, n, clo, csz) in enumerate(plan):
                    xt = xh.tile([128, 2, csz], BF16, tag=f"c{j}")
                    gather_chunk(b, coloff, csz, xt)
                    coloff += csz // 16
                    xs.append(xt)

                H = hp.tile([128, 2, SO], BF16, tag="H")
                zp = pp.tile([L, npl], FP32, tag="zp")   # partial Z
                np_ = pp.tile([L, npl], FP32, tag="np")  # partial num

                def score_tile(j):
                    """Combined scores/t matmul for H tile j and the
                    softmax partials for that tile. Scores are bounded
                    (|score| <= |U_l|·|H_s| ~ 2.6, H in [-1,1]) so raw
                    exp in fp32 needs no max subtraction."""
                    t0, n, _, _ = plan[j]
                    pst = psum_st.tile([114, TW], FP32, tag="st")
                    for fc in range(2):
                        last_pe[0] = nc.tensor.matmul(
                            pst[:, 0:n], uw_sb[:, fc, :], H[:, fc, t0:t0 + n],
                            start=(fc == 0), stop=(fc == 1),
                        )
                    e_sb = ep.tile([L, TW], FP32, tag="e")
                    nc.scalar.activation(
                        e_sb[:, 0:n], pst[0:L, 0:n],
                        mybir.ActivationFunctionType.Exp,
                        accum_out=zp[:, j:j + 1],
                    )
                    nc.vector.tensor_mul(e_sb[:, 0:n], e_sb[:, 0:n],
                                         pst[64:64 + L, 0:n])
                    nc.vector.reduce_sum(np_[:, j:j + 1], e_sb[:, 0:n],
                                         axis=mybir.AxisListType.X)

                # ---- conv1d(k=4) + bias + tanh; scores lag one tile ----
                for j, (t0, n, clo, csz) in enumerate(plan):
                    shifts = []
                    for k in range(4):
                        lo = max(0, t0 + k - 2)
                        hi = min(S, t0 + k - 2 + n)
                        shifts.append((k, lo, hi, lo - (t0 + k - 2)))
                    shifts.sort(key=lambda s: -(s[2] - s[1]))
                    for fc in range(2):
                        ph = psum.tile([128, TW], FP32, tag=f"h{fc}")
                        mms = [(k, lo, hi, off, ec)
                               for (k, lo, hi, off) in shifts
                               for ec in range(2)]
                        for i, (k, lo, hi, off, ec) in enumerate(mms):
                            rel = lo - clo
                            mi = nc.tensor.matmul(
                                ph[:, off:off + (hi - lo)],
                                w_sb[:, k * 4 + ec * 2 + fc, :],
                                xs[j][:, ec, rel:rel + (hi - lo)],
                                start=(i == 0), stop=(i == len(mms) - 1),
                            )
                            if i == 0 and fc == 0:
                                # explicit RAW gate, attached directly to
                                # the first matmul of the tile: PE blocks
                                # until chunk (b, j)'s gather DMA fully
                                # landed (16 incs = all 16 SDMA engines)
                                mi._wait_ge(gsems[gbase + j], 16)
                            last_pe[0] = mi
                        nc.scalar.activation(
                            H[:, fc, t0:t0 + n], ph[:, 0:n],
                            mybir.ActivationFunctionType.Tanh,
                            bias=cb_sb[:, fc:fc + 1],
                        )
                    if j > 0:
                        score_tile(j - 1)
                score_tile(npl - 1)

                # ---- combine partials -> logits ----
                zsum = small.tile([L, 1], FP32, tag="zsum")
                nc.vector.reduce_sum(zsum[:], zp[:], axis=mybir.AxisListType.X)
                nsum = small.tile([L, 1], FP32, tag="nsum")
                nc.vector.reduce_sum(nsum[:], np_[:], axis=mybir.AxisListType.X)
                zr = small.tile([L, 1], FP32, tag="zr")
                nc.vector.reciprocal(zr[:], zsum[:])
                sm = small.tile([L, 1], FP32, tag="sm")
                nc.vector.tensor_mul(sm[:], nsum[:], zr[:])
                nc.vector.tensor_add(out_sb[:, b:b + 1], sm[:], fb_sb[:])

            nc.sync.dma_start(out_ap[:], out_sb[:])

    nc.compile()
    return nc


def _prep_shared(emb_table, conv_w, conv_b, U_w, final_w, final_b):
    emb_bf = np.ascontiguousarray(emb_table.astype(ml_dtypes.bfloat16))

    # wconv[e_lo, k*4 + ec*2 + fc, f_lo] = conv_w[fc*128+f, ec*128+e, k]
    W = np.empty((128, 16, 128), np.float32)
    for k in range(4):
        for ec in range(2):
            for fc in range(2):
                W[:, k * 4 + ec * 2 + fc, :] = conv_w[
                    fc * 128:(fc + 1) * 128, ec * 128:(ec + 1) * 128, k].T
    W = np.ascontiguousarray(W.astype(ml_dtypes.bfloat16))

    # uwfw[f_lo, fc, j]: j<50 -> U_w[j, fc*128+f_lo];
    # j in [64,114) -> final_w[j-64, fc*128+f_lo]; rest zero
    UW = np.zeros((128, 2, 114), np.float32)
    UW[:, :, 0:L] = U_w.T.reshape(2, 128, L).transpose(1, 0, 2)
    UW[:, :, 64:64 + L] = final_w.T.reshape(2, 128, L).transpose(1, 0, 2)
    UW = np.ascontiguousarray(UW.astype(ml_dtypes.bfloat16))

    CB = np.ascontiguousarray(conv_b.reshape(2, 128).T.astype(np.float32))
    FB = np.ascontiguousarray(final_b.reshape(L, 1).astype(np.float32))
    return emb_bf, W, UW, CB, FB


def _build_idx(cid):
    """(BPC, S) int16 -> (128, BPC*IPB) gather index table."""
    cols = []
    for b in range(BPC):
        for (t0, n, clo, csz) in _plan(b):
            v = cid[b, clo:clo + csz]
            cols.append(v.reshape(csz // 16, 16).T)
    blk = np.concatenate(cols, axis=1)  # (16, BPC*IPB)
    return np.ascontiguousarray(np.tile(blk, (8, 1)))


def kernel(input_ids, emb_table, conv_w, conv_b, U_w, final_w, final_b):
    import os
    ids = np.asarray(input_ids)
    emb_table = np.asarray(emb_table, dtype=np.float32)
    conv_w = np.asarray(conv_w, dtype=np.float32)
    conv_b = np.asarray(conv_b, dtype=np.float32)
    U_w = np.asarray(U_w, dtype=np.float32)
    final_w = np.asarray(final_w, dtype=np.float32)
    final_b = np.asarray(final_b, dtype=np.float32)

    if "nc" not in _cache:
        _cache["nc"] = build_nc()
    nc = _cache["nc"]

    emb_bf, W, UW, CB, FB = _prep_shared(
        emb_table, conv_w, conv_b, U_w, final_w, final_b)

    ids16 = ids.astype(np.int16)  # vocab 30522 < 2**15
    in_maps = []
    for c in range(N_CORES):
        idx = _build_idx(ids16[c * BPC:(c + 1) * BPC])
        in_maps.append({
            "emb": emb_bf, "idx": idx,
            "wconv": W, "uwfw": UW, "cbias": CB, "fbias": FB,
        })

    trace = bool(int(os.environ.get("KERNEL_TRACE", "0")))
    res = run_bass_kernel_spmd(nc, in_maps, core_ids=list(range(N_CORES)),
                               trace=trace)
    _cache["last_result"] = res

    out = np.concatenate(
        [res.results[c]["out"].T for c in range(N_CORES)], axis=0)
    return np.ascontiguousarray(out.astype(np.float32))


# revision 30
# speedup vs baseline: 1.3664x; 1.0021x over previous
"""Trainium2 Bass kernel for nn_CAML_53240414601378.

Embedding lookup -> Conv1d(k=4, pad=2) -> tanh -> per-label attention
pooling -> logits. Data-parallel over batch across 8 NeuronCores
(4 batches per core); small params replicated.

Final pipeline design (from perfetto analysis; 196us -> 159us):
- Conv tiles are ~508 output columns wide so each tile's x-window
  [t0-2, t0+510) is exactly one 512-index gather chunk. Every conv
  matmul group depends on ONE gather chunk, so the PE starts as soon
  as the first chunk lands instead of waiting for a whole batch.
  Batch 0 uses a small 128-index first chunk (124-col first tile) to
  start the PE even earlier.
- Gathers run on 4 SWDGE queues. Tile assigns DMA-completion sem
  lanes round-robin over the 8 DMASW lanes in *scheduled* order with
  cumulative thresholds, which silently assumes per-lane in-order
  completion; with multiple queues that only holds if lane index and
  queue index stay congruent. All 36 gathers are therefore chained
  with ordering-only deps (pinning scheduled order = emission order)
  and queue = position % 4, so lane g%8 always serves queue g%4.
- Gathers use single_packet=True: with multi-packet chains the
  gather's completion semaphore was observed firing before all data
  packets landed (corrupted batches whenever the PE consumed a chunk
  right at sem-fire time). Single-packet chains order the per-engine
  sem descriptor strictly after that engine's data.
- Explicit per-chunk semaphores (then_inc 16) + _wait_ge on the first
  matmul of each tile make the gather->conv RAW edge unconditional;
  Tile's auto-generated waits are pruned against its scheduling-sim
  timeline, which under-synchronizes multi-queue SWDGE on hardware.
- Warmup matmuls (consumed by a dummy reduce) run during the initial
  gather latency so the PE HAM clock gate is already at 8/8 when the
  real conv stream starts.
- scores matmul for tile j-1 is emitted after conv tile j so the PE
  never waits on the tanh activation of the tile it just produced.
- softmax runs without max-subtraction (scores provably bounded by
  ~2.6 here, fp32 exp is safe), shortening the per-tile DVE work and
  the exposed final combine chain.
"""

import numpy as np
import ml_dtypes

import concourse.bass as bass
import concourse.tile as tile
from concourse.tile import add_dep_helper
from concourse import bacc, mybir
from concourse.bass_utils import run_bass_kernel_spmd

B, S = 32, 4096
VOCAB, E, F, L = 30522, 256, 256, 50
SO = S + 1  # conv output length (4097)
N_CORES = 8
BPC = B // N_CORES  # batches per core
BF16 = mybir.dt.bfloat16
FP32 = mybir.dt.float32

TW = 508                      # max conv tile width (outputs per tile)
NJ = 9                        # tiles per batch
NQ = 4                        # SWDGE queues for the gathers


def _plan(b):
    """Per-batch tile plan: list of (t0, n, chunk_lo, chunk_sz)."""
    if b == 0:
        # small 128-idx chunks on all 4 queues first: the PE starts on
        # tile 0 ~4us sooner and never stalls on the gather ramp
        p = [(124 * i, 124, max(0, 124 * i - 2), 128) for i in range(4)]
        for i in range(7):
            p.append((496 + 508 * i, 508, 494 + 508 * i, 512))
        p.append((4052, 45, S - 128, 128))
        return p
    clo = [0] + [TW * j - 2 for j in range(1, 8)] + [S - 128]
    csz = [512] * 8 + [128]
    return [(TW * j, min(SO - TW * j, TW), clo[j], csz[j]) for j in range(NJ)]


IPB = sum(c[3] for c in _plan(1)) // 16  # idx cols per batch (264)
N_CHUNKS = len(_plan(0)) + (BPC - 1) * len(_plan(1))

_cache = {}


def build_nc():
    nc = bacc.Bacc("TRN2", target_bir_lowering=False, debug=False,
                   num_devices=N_CORES, num_swdge_queues=NQ)

    # One semaphore per gather chunk, allocated before the TileContext so
    # they sit in a contiguous range below the tile framework's own sems.
    # Tile's auto-generated DMA waits are pruned against the *scheduling
    # sim's* timeline, which models SWDGE DMA as serialized; on hardware
    # the 4-queue gathers complete later relative to the PE stream and the
    # pruned waits let conv matmuls read chunks before the DMA lands
    # (observed as corrupted early batches). Explicit per-chunk sems +
    # tensor-engine wait_ge gates make the RAW edge unconditional; one
    # sem per chunk (16 incs, one per SDMA engine) is also safe against
    # inter-engine skew, unlike cumulative per-lane thresholds.
    gsems = [nc.alloc_semaphore(f"gch{g}") for g in range(N_CHUNKS)]
    gsem_lo = min(s.num for s in gsems)
    gsem_hi = max(s.num for s in gsems)

    emb_ap = nc.dram_tensor("emb", (VOCAB, E), BF16, kind="ExternalInput").ap()
    idx_ap = nc.dram_tensor("idx", (128, BPC * IPB), mybir.dt.int16,
                            kind="ExternalInput").ap()
    w_ap = nc.dram_tensor("wconv", (128, 16, 128), BF16,
                          kind="ExternalInput").ap()
    uw_ap = nc.dram_tensor("uwfw", (128, 2, 114), BF16,
                           kind="ExternalInput").ap()
    cb_ap = nc.dram_tensor("cbias", (128, 2), FP32, kind="ExternalInput").ap()
    fb_ap = nc.dram_tensor("fbias", (L, 1), FP32, kind="ExternalInput").ap()
    out_ap = nc.dram_tensor("out", (L, BPC), FP32, kind="ExternalOutput").ap()

    with tile.TileContext(nc) as tc:
        with (
            tc.tile_pool(name="const", bufs=1) as const,
            # 4 bufs: each batch gets its own buffer per chunk tag, so no
            # gather ever overwrites a buffer an earlier batch still reads
            # (WAR edges across batches disappear structurally)
            tc.tile_pool(name="xh", bufs=4) as xh,      # gather chunk tiles
            tc.tile_pool(name="hp", bufs=2) as hp,
            tc.tile_pool(name="ep", bufs=3) as ep,      # exp scratch tiles
            tc.tile_pool(name="pp", bufs=2) as pp,      # per-batch partials
            tc.tile_pool(name="small", bufs=8) as small,
            tc.tile_pool(name="psum", bufs=2, space="PSUM") as psum,
            tc.tile_pool(name="psum_st", bufs=4, space="PSUM") as psum_st,
        ):
            # ---- constants; idx first so gathers start ASAP. The first
            # 32 idx cols (batch 0's four 128-idx chunks) ride a tiny
            # separate DMA so their gathers launch ~2us earlier ----
            idx_a = const.tile([128, 32], mybir.dt.int16)
            nc.sync.dma_start(idx_a[:], idx_ap[:, 0:32])
            idx_sb = const.tile([128, BPC * IPB - 32], mybir.dt.int16)
            nc.sync.dma_start(idx_sb[:], idx_ap[:, 32:])
            w_sb = const.tile([128, 16, 128], BF16)
            nc.sync.dma_start(w_sb[:], w_ap[:])
            uw_sb = const.tile([128, 2, 114], BF16)
            nc.sync.dma_start(uw_sb[:], uw_ap[:])
            cb_sb = const.tile([128, 2], FP32)
            nc.sync.dma_start(cb_sb[:], cb_ap[:])
            fb_sb = const.tile([L, 1], FP32)
            nc.sync.dma_start(fb_sb[:], fb_ap[:])
            out_sb = const.tile([L, BPC], FP32)

            r512 = nc.gpsimd.to_reg(512)
            r128 = nc.gpsimd.to_reg(128)

            # sems are not zeroed on alloc; clear before any gather can inc
            clr = nc.gpsimd.sem_clear(range(gsem_lo, gsem_hi + 1))

            # ---- HAM warmup: ~3.5us of throwaway matmuls so the PE
            # clock gate is 8/8 by the time the first chunk lands ----
            warm_rhs = idx_sb[:, 0:TW].bitcast(BF16)
            wp = psum_st.tile([114, TW], FP32, tag="st")
            last_pe = [None]
            for i in range(16):
                last_pe[0] = nc.tensor.matmul(
                    wp[:, 0:TW], w_sb[:, i % 16, 0:114], warm_rhs,
                    start=(i == 0), stop=(i == 15))
            warm_out = small.tile([L, 1], FP32, tag="warm")
            nc.vector.reduce_max(warm_out[:], wp[0:L, :],
                                 axis=mybir.AxisListType.X)

            prev_g = [None]  # last gather emitted (global chain)
            gcnt = [0]

            def gather_chunk(b, coloff, csz, xs):
                g = gcnt[0]
                q = g % NQ
                gcnt[0] += 1
                c0 = b * IPB + coloff
                if c0 < 32:
                    iap = idx_a[:, c0:c0 + csz // 16]
                else:
                    iap = idx_sb[:, c0 - 32:c0 - 32 + csz // 16]
                gi = nc.gpsimd.dma_gather(
                    out_ap=xs[:], in_ap=emb_ap[:],
                    idxs_ap=iap,
                    num_idxs=csz, num_idxs_reg=(r512 if csz == 512 else r128),
                    elem_size=E, transpose=True, single_packet=True,
                    queue_num=q)
                gi.then_inc(gsems[g], 16)
                if prev_g[0] is None:
                    add_dep_helper(gi.ins, clr.ins, False, "sems cleared")
                else:
                    # ordering-only dep: pins scheduled order = emission
                    # order so DMASW lane g%8 always serves queue g%4
                    add_dep_helper(gi.ins, prev_g[0].ins, False,
                                   "gather lane/queue alignment")
                prev_g[0] = gi

            for b in range(BPC):
                plan = _plan(b)
                npl = len(plan)
                gbase = gcnt[0]
                xs = []
                coloff = 0
                for j, (t0, n, clo, csz) in enumerate(plan):
                    xt = xh.tile([128, 2, csz], BF16, tag=f"c{j}")
                    gather_chunk(b, coloff, csz, xt)
                    coloff += csz // 16
                    xs.append(xt)

                H = hp.tile([128, 2, SO], BF16, tag="H")
                zp = pp.tile([L, npl], FP32, tag="zp")   # partial Z
                np_ = pp.tile([L, npl], FP32, tag="np")  # partial num

                def score_tile(j):
                    """Combined scores/t matmul for H tile j and the
                    softmax partials for that tile. Scores are bounded
                    (|score| <= |U_l|·|H_s| ~ 2.6, H in [-1,1]) so raw
                    exp in fp32 needs no max subtraction."""
                    t0, n, _, _ = plan[j]
                    pst = psum_st.tile([114, TW], FP32, tag="st")
                    for fc in range(2):
                        last_pe[0] = nc.tensor.matmul(
                            pst[:, 0:n], uw_sb[:, fc, :], H[:, fc, t0:t0 + n],
                            start=(fc == 0), stop=(fc == 1),
                        )
                    e_sb = ep.tile([L, TW], FP32, tag="e")
                    nc.scalar.activation(
                        e_sb[:, 0:n], pst[0:L, 0:n],
                        mybir.ActivationFunctionType.Exp,
                        accum_out=zp[:, j:j + 1],
                    )
                    nc.vector.tensor_mul(e_sb[:, 0:n], e_sb[:, 0:n],
                                         pst[64:64 + L, 0:n])
                    nc.vector.reduce_sum(np_[:, j:j + 1], e_sb[:, 0:n],
                                         axis=mybir.AxisListType.X)

                # ---- conv1d(k=4) + bias + tanh; scores lag one tile ----
                for j, (t0, n, clo, csz) in enumerate(plan):
                    shifts = []
                    for k in range(4):
                        lo = max(0, t0 + k - 2)
                        hi = min(S, t0 + k - 2 + n)
                        shifts.append((k, lo, hi, lo - (t0 + k - 2)))
                    shifts.sort(key=lambda s: -(s[2] - s[1]))
                    for fc in range(2):
                        ph = psum.tile([128, TW], FP32, tag=f"h{fc}")
                        mms = [(k, lo, hi, off, ec)
                               for (k, lo, hi, off) in shifts
                               for ec in range(2)]
                        for i, (k, lo, hi, off, ec) in enumerate(mms):
                            rel = lo - clo
                            mi = nc.tensor.matmul(
                                ph[:, off:off + (hi - lo)],
                                w_sb[:, k * 4 + ec * 2 + fc, :],
                                xs[j][:, ec, rel:rel + (hi - lo)],
                                start=(i == 0), stop=(i == len(mms) - 1),
                            )
                            if i == 0 and fc == 0:
                                # explicit RAW gate, attached directly to
                                # the first matmul of the tile: PE blocks
                                # until chunk (b, j)'s gather DMA fully
                                # landed (16 incs = all 16 SDMA engines)
                                mi._wait_ge(gsems[gbase + j], 16)
                            last_pe[0] = mi
                        nc.scalar.activation(
                            H[:, fc, t0:t0 + n], ph[:, 0:n],
                            mybir.ActivationFunctionType.Tanh,
                            bias=cb_sb[:, fc:fc + 1],
                        )
                    if j > 0:
                        score_tile(j - 1)
                score_tile(npl - 1)

                # ---- combine partials -> logits ----
                zsum = small.tile([L, 1], FP32, tag="zsum")
                nc.vector.reduce_sum(zsum[:], zp[:], axis=mybir.AxisListType.X)
                nsum = small.tile([L, 1], FP32, tag="nsum")
                nc.vector.reduce_sum(nsum[:], np_[:], axis=mybir.AxisListType.X)
                zr = small.tile([L, 1], FP32, tag="zr")
                nc.vector.reciprocal(zr[:], zsum[:])
                sm = small.tile([L, 1], FP32, tag="sm")
                nc.vector.tensor_mul(sm[:], nsum[:], zr[:])
                nc.vector.tensor_add(out_sb[:, b:b + 1], sm[:], fb_sb[:])

            nc.sync.dma_start(out_ap[:], out_sb[:])

    nc.compile()
    return nc


def _prep_shared(emb_table, conv_w, conv_b, U_w, final_w, final_b):
    emb_bf = np.ascontiguousarray(emb_table.astype(ml_dtypes.bfloat16))

    # wconv[e_lo, k*4 + ec*2 + fc, f_lo] = conv_w[fc*128+f, ec*128+e, k]
    W = np.empty((128, 16, 128), np.float32)
    for k in range(4):
        for ec in range(2):
            for fc in range(2):
                W[:, k * 4 + ec * 2 + fc, :] = conv_w[
                    fc * 128:(fc + 1) * 128, ec * 128:(ec + 1) * 128, k].T
    W = np.ascontiguousarray(W.astype(ml_dtypes.bfloat16))

    # uwfw[f_lo, fc, j]: j<50 -> U_w[j, fc*128+f_lo];
    # j in [64,114) -> final_w[j-64, fc*128+f_lo]; rest zero
    UW = np.zeros((128, 2, 114), np.float32)
    UW[:, :, 0:L] = U_w.T.reshape(2, 128, L).transpose(1, 0, 2)
    UW[:, :, 64:64 + L] = final_w.T.reshape(2, 128, L).transpose(1, 0, 2)
    UW = np.ascontiguousarray(UW.astype(ml_dtypes.bfloat16))

    CB = np.ascontiguousarray(conv_b.reshape(2, 128).T.astype(np.float32))
    FB = np.ascontiguousarray(final_b.reshape(L, 1).astype(np.float32))
    return emb_bf, W, UW, CB, FB


def _build_idx(cid):
    """(BPC, S) int16 -> (128, BPC*IPB) gather index table."""
    cols = []
    for b in range(BPC):
        for (t0, n, clo, csz) in _plan(b):
            v = cid[b, clo:clo + csz]
            cols.append(v.reshape(csz // 16, 16).T)
    blk = np.concatenate(cols, axis=1)  # (16, BPC*IPB)
    return np.ascontiguousarray(np.tile(blk, (8, 1)))


def kernel(input_ids, emb_table, conv_w, conv_b, U_w, final_w, final_b):
    import os
    ids = np.asarray(input_ids)
    emb_table = np.asarray(emb_table, dtype=np.float32)
    conv_w = np.asarray(conv_w, dtype=np.float32)
    conv_b = np.asarray(conv_b, dtype=np.float32)
    U_w = np.asarray(U_w, dtype=np.float32)
    final_w = np.asarray(final_w, dtype=np.float32)
    final_b = np.asarray(final_b, dtype=np.float32)

    if "nc" not in _cache:
        _cache["nc"] = build_nc()
    nc = _cache["nc"]

    emb_bf, W, UW, CB, FB = _prep_shared(
        emb_table, conv_w, conv_b, U_w, final_w, final_b)

    ids16 = ids.astype(np.int16)  # vocab 30522 < 2**15
    in_maps = []
    for c in range(N_CORES):
        idx = _build_idx(ids16[c * BPC:(c + 1) * BPC])
        in_maps.append({
            "emb": emb_bf, "idx": idx,
            "wconv": W, "uwfw": UW, "cbias": CB, "fbias": FB,
        })

    trace = bool(int(os.environ.get("KERNEL_TRACE", "0")))
    res = run_bass_kernel_spmd(nc, in_maps, core_ids=list(range(N_CORES)),
                               trace=trace)
    _cache["last_result"] = res

    out = np.concatenate(
        [res.results[c]["out"].T for c in range(N_CORES)], axis=0)
    return np.ascontiguousarray(out.astype(np.float32))
